# revision 1
# baseline (speedup 1.0000x reference)
"""Fused pre-LN multi-head attention kernel for Trainium2 (8 NeuronCores).

Problem: B=4, S=2048, D=256, H=8, KD=256.
    out = x_q + MHA(LN(x_q), LN(x_k), LN(x_v))   (keras-style, q scaled 1/sqrt(KD))

Sharding: core c -> batch b = c//2, head group hg = c%2 (4 heads each).
Each core runs a fused flash-style attention over its (batch, 4 heads) and
produces per-head UNNORMALIZED transposed partial outputs plus the softmax
denominators. Host folds LN gamma/beta and all biases into the projection
weights, gathers the 8 cores' partials, divides by the denominators (the
division commutes with the output projection, which contracts kd, not q),
sums heads, and adds residual + constant bias terms.

Device dataflow (all-transposed to keep reductions on friendly axes):
  1. LN stats in natural layout [S,D] (bn_stats/bn_aggr, per-partition ops),
     xhat=(x-mu)*rsqrt(var+eps) cast to bf16.
  2. PE-transpose xhat -> xhatT [D, S].
  3. Projections on PE: qT,kT = W_eff^T @ xhatT (q bias via ACT Identity),
     v natural = xhat @ Wv_eff.
  4. Per (head, 512-wide q-block): scoresT[k,q] accumulated in PSUM,
     exp on ACT (scores bounded ~|7| here, so no max subtraction) -> bf16
     probs; denominator via ones-matmul into PSUM; PV matmul -> attnT
     (unnormalized, bf16 in SBUF).
  5. Output projection per head (PE) -> fp32 staging -> DMA out per head.
ACT runs exactly two table sets for the whole kernel (sqrt, then exp).
"""

import numpy as np
import ml_dtypes

import concourse.bass as bass
import concourse.bacc as bacc
import concourse.mybir as mybir
import concourse.tile as tile
from concourse.bass_utils import run_bass_kernel_spmd
from concourse.masks import make_identity

B, S, D, H, KD = 4, 2048, 256, 8, 256
HPC = 4              # heads per core
NCORES = 8
EPS = 1e-5
FP = mybir.dt.float32
BF = mybir.dt.bfloat16
AF = mybir.ActivationFunctionType
ALU = mybir.AluOpType

M_HKD = HPC * KD // 128      # 8   qT/kT partition chunks
N_S = S // 512               # 4   512-wide column blocks of S
KB = S // 128                # 16  key-position chunks
DC = D // 128                # 2   contraction / d chunks


def build_program() -> bass.Bass:
    nc = bacc.Bacc(None)

    xq = nc.declare_dram_parameter("xq", [S, D], FP, isOutput=False)
    xk = nc.declare_dram_parameter("xk", [S, D], FP, isOutput=False)
    xv = nc.declare_dram_parameter("xv", [S, D], FP, isOutput=False)
    wq = nc.declare_dram_parameter("wq", [128, DC, HPC * KD], BF, isOutput=False)
    wk = nc.declare_dram_parameter("wk", [128, DC, HPC * KD], BF, isOutput=False)
    wv = nc.declare_dram_parameter("wv", [128, DC, HPC * KD], BF, isOutput=False)
    wo = nc.declare_dram_parameter("wo", [128, HPC, DC, D], BF, isOutput=False)
    bqt = nc.declare_dram_parameter("bqt", [128, M_HKD], FP, isOutput=False)
    out_d = nc.declare_dram_parameter("outT", [HPC, 128, DC, S], FP, isOutput=True)
    den_d = nc.declare_dram_parameter("dens", [HPC * N_S, 512], FP, isOutput=True)

    with tile.TileContext(nc) as tc:
        with (
            tc.tile_pool(name="consts", bufs=1) as consts,
            tc.tile_pool(name="persist", bufs=1) as persist,
            tc.tile_pool(name="probs", bufs=6) as probs_p,
            tc.tile_pool(name="attn_n", bufs=2) as attn_p,
            tc.tile_pool(name="outstage", bufs=1) as out_p,
            tc.tile_pool(name="denstage", bufs=2) as den_p,
        ):
            # ---- constants ----
            ident = consts.tile([128, 128], BF)
            make_identity(nc, ident)
            ones_den = consts.tile([128, 1], BF)
            nc.vector.memset(ones_den, 1.0)
            eps_t = consts.tile([128, 1], FP)
            nc.vector.memset(eps_t, EPS)

            # ---- weights ----
            wq_t = consts.tile([128, DC, HPC * KD], BF)
            wk_t = consts.tile([128, DC, HPC * KD], BF)
            wv_t = consts.tile([128, DC, HPC * KD], BF)
            wo_t = consts.tile([128, HPC, DC, D], BF)
            bqt_t = consts.tile([128, M_HKD], FP)
            def load_weights():
                # Emitted after the q-stream x loads so the input DMAs the
                # LN chain needs are first in the queue. Exactly 8 DMAs to
                # keep the HWDGE lane round-robin aligned with the 8-deep
                # x-load slot recycling.
                nc.sync.dma_start(out=wq_t, in_=wq[:])
                nc.sync.dma_start(out=wk_t, in_=wk[:])
                nc.sync.dma_start(out=wv_t, in_=wv[:])
                nc.sync.dma_start(out=wo_t, in_=wo[:])
                for bi in range(4):
                    nc.sync.dma_start(out=bqt_t[:, bi * 2:(bi + 1) * 2],
                                      in_=bqt[:, bi * 2:(bi + 1) * 2])

            # ---- persistent big tensors ----
            qT = persist.tile([128, M_HKD, S], BF, tag="qT")
            kT = persist.tile([128, M_HKD, S], BF, tag="kT")
            v_t = persist.tile([128, KB, HPC * KD], BF, tag="v")

            # Phase A-C pools live in a nested scope so their SBUF zone is
            # released before the attention phase allocates its buffers.
            with (
                tc.tile_pool(name="xraw", bufs=2) as xraw_p,
                tc.tile_pool(name="ln_small", bufs=8) as ln_small,
                tc.tile_pool(name="xhat", bufs=4) as xhat_p,
                tc.tile_pool(name="xhatT", bufs=1) as xhatT_p,
                tc.tile_pool(name="ps_ac", bufs=6,
                             space=bass.MemorySpace.PSUM) as ps_ac,
            ):
                xhatT = {
                    t: xhatT_p.tile([128, DC, S], BF, name=f"xhatT_{t}",
                                    tag=f"xhatT_{t}")
                    for t in ("q", "k", "v")
                }
                # ---- Phase A+B: LayerNorm + transpose, per stream ----
                for sidx, (tname, xdram) in enumerate(
                        (("k", xk), ("q", xq), ("v", xv))):
                    if sidx == 1:
                        load_weights()
                    xre = xdram.rearrange("(c t p) d -> c p t d", t=8, p=128)
                    xr = None
                    for i in range(KB):
                        c, t = divmod(i, 8)
                        if t == 0:
                            xr = xraw_p.tile([128, 8, D], FP, tag="xraw")
                            nc.sync.dma_start(out=xr, in_=xre[c])
                        x_i = xr[:, t, :]
                        stats = ln_small.tile([128, 6], FP, tag="stats")
                        nc.vector.bn_stats(out=stats, in_=x_i)
                        mv = ln_small.tile([128, 2], FP, tag="mv")
                        nc.vector.bn_aggr(out=mv, in_=stats)
                        sd = ln_small.tile([128, 1], FP, tag="sd")
                        # sd = sqrt(var + eps)
                        nc.scalar.activation(
                            out=sd, in_=mv[:, 1:2], func=AF.Sqrt,
                            bias=eps_t[:], scale=1.0,
                        )
                        r = ln_small.tile([128, 1], FP, tag="r")
                        nc.vector.reciprocal(out=r, in_=sd)
                        negmr = ln_small.tile([128, 1], FP, tag="negmr")
                        nc.vector.tensor_scalar(
                            out=negmr, in0=mv[:, 0:1],
                            scalar1=r, scalar2=-1.0,
                            op0=ALU.mult, op1=ALU.mult,
                        )
                        xh = xhat_p.tile([128, D], BF, tag="xh")
                        nc.scalar.activation(
                            out=xh, in_=x_i, func=AF.Identity,
                            bias=negmr[:], scale=r[:],
                        )
                        for j in range(DC):
                            pst = ps_ac.tile([128, 128], BF, name="pst", tag="ac")
                            nc.tensor.transpose(
                                pst, xh[:, j * 128:(j + 1) * 128], ident)
                            dst_sl = xhatT[tname][:, j, i * 128:(i + 1) * 128]
                            if j == 0:
                                nc.vector.tensor_copy(out=dst_sl, in_=pst)
                            else:
                                nc.scalar.copy(out=dst_sl, in_=pst)

                # ---- Phase C: projections ----
                # qT/kT: [HPC*KD, S] = W_eff^T @ xhatT
                for dst, w_t, src, biased in (
                    (kT, wk_t, xhatT["k"], False),
                    (qT, wq_t, xhatT["q"], True),
                ):
                    for m in range(M_HKD):
                        for n in range(N_S):
                            ps = ps_ac.tile([128, 512], FP, tag="ac")
                            for kd in range(DC):
                                nc.tensor.matmul(
                                    ps,
                                    w_t[:, kd, m * 128:(m + 1) * 128],
                                    src[:, kd, n * 512:(n + 1) * 512],
                                    start=(kd == 0), stop=(kd == DC - 1),
                                )
                            dsl = dst[:, m, n * 512:(n + 1) * 512]
                            if biased:
                                nc.scalar.activation(
                                    out=dsl, in_=ps, func=AF.Identity,
                                    bias=bqt_t[:, m:m + 1], scale=1.0)
                            elif n % 2 == 0:
                                nc.scalar.copy(out=dsl, in_=ps)
                            else:
                                nc.vector.tensor_copy(out=dsl, in_=ps)
                # v natural: [S, HPC*KD] = xhat @ Wv_eff
                for i in range(KB):
                    for n in range(HPC * KD // 512):
                        ps = ps_ac.tile([128, 512], FP, tag="ac")
                        for kd in range(DC):
                            nc.tensor.matmul(
                                ps,
                                xhatT["v"][:, kd, i * 128:(i + 1) * 128],
                                wv_t[:, kd, n * 512:(n + 1) * 512],
                                start=(kd == 0), stop=(kd == DC - 1),
                            )
                        if n % 2 == 0:
                            nc.scalar.copy(
                                out=v_t[:, i, n * 512:(n + 1) * 512], in_=ps)
                        else:
                            nc.vector.tensor_copy(
                                out=v_t[:, i, n * 512:(n + 1) * 512], in_=ps)

            # ---- Phase D: attention per (head, q-block) ----
            # attnT and outT stay UNNORMALIZED on device; the softmax
            # denominator commutes with the output projection (it contracts
            # kd; each q column is independent) -> host divides.
            # PSUM: ps_s (4 x 1 bank) + ps_pv (2 banks) + ps_den (1) +
            # ps_po (1) = 8, allocated after the phase A-C pool releases.
            phase_d_pools = (
                tc.tile_pool(name="ps_s", bufs=4, space=bass.MemorySpace.PSUM),
                tc.tile_pool(name="ps_pv", bufs=1, space=bass.MemorySpace.PSUM),
                tc.tile_pool(name="ps_den", bufs=1, space=bass.MemorySpace.PSUM),
                tc.tile_pool(name="ps_po", bufs=1, space=bass.MemorySpace.PSUM),
            )
            ps_s = phase_d_pools[0].__enter__()
            ps_pv = phase_d_pools[1].__enter__()
            ps_den = phase_d_pools[2].__enter__()
            ps_po = phase_d_pools[3].__enter__()
            for h in range(HPC):
                attnT_u = attn_p.tile([128, DC, S], BF, tag="attnT")
                o_t = out_p.tile([128, DC, S], FP, tag="o")
                for qb in range(N_S):
                    pv_t = ps_pv.tile([128, DC * 512], FP, name="pv_t", tag="pv")
                    pv = [pv_t[:, m * 512:(m + 1) * 512] for m in range(DC)]
                    den = ps_den.tile([1, 512], FP, tag="den")
                    # Software-pipelined by two kb: scores(kb) and exp(kb)
                    # are emitted two iterations ahead of den/PV(kb) so the
                    # in-order PE queue never stalls on exp latency.
                    prs = [None] * KB
                    pairs = [None] * (KB // 2)
                    for kb in range(KB + 2):
                        if kb < KB:
                            ss = ps_s.tile([128, 512], FP, tag="ss")
                            for kd in range(DC):
                                nc.tensor.matmul(
                                    ss,
                                    kT[:, DC * h + kd, kb * 128:(kb + 1) * 128],
                                    qT[:, DC * h + kd, qb * 512:(qb + 1) * 512],
                                    start=(kd == 0), stop=(kd == DC - 1),
                                )
                            pr = probs_p.tile([128, 512], BF, tag="pr")
                            nc.scalar.activation(out=pr, in_=ss, func=AF.Exp)
                            prs[kb] = pr
                        if kb >= 2:
                            kp = kb - 2
                            for m in range(DC):
                                nc.tensor.matmul(
                                    pv[m],
                                    v_t[:, kp, h * KD + m * 128: h * KD + (m + 1) * 128],
                                    prs[kp],
                                    start=(kp == 0), stop=(kp == KB - 1),
                                )
                            if kp % 2 == 1:
                                # quarter the PE cost of the denominator:
                                # pair- then quad-sum probs on DVE (fp32
                                # internal adds), one ones-matmul per quad
                                prsum = probs_p.tile([128, 512], BF, name="prsum",
                                                     tag="prsum", bufs=3)
                                nc.vector.tensor_add(
                                    prsum, prs[kp - 1], prs[kp])
                                pairs[(kp - 1) // 2] = prsum
                                prs[kp - 1] = None
                                prs[kp] = None
                            if kp % 4 == 3:
                                qsum = probs_p.tile([128, 512], BF, name="qsum",
                                                    tag="qsum", bufs=2)
                                nc.vector.tensor_add(
                                    qsum, pairs[kp // 2 - 1], pairs[kp // 2])
                                nc.tensor.matmul(
                                    den, ones_den, qsum,
                                    start=(kp == 3), stop=(kp == KB - 1),
                                )
                                pairs[kp // 2 - 1] = None
                                pairs[kp // 2] = None
                    ds_t = den_p.tile([1, 512], FP, tag="ds")
                    nc.vector.tensor_copy(out=ds_t, in_=den)
                    nc.sync.dma_start(out=den_d[h * N_S + qb], in_=ds_t)
                    for m in range(DC):
                        nc.vector.tensor_copy(
                            out=attnT_u[:, m, qb * 512:(qb + 1) * 512], in_=pv[m],
                        )
                    # ---- output projection for this q-block ----
                    for dc in range(DC):
                        po = ps_po.tile([128, 512], FP, tag="po")
                        for kd in range(DC):
                            nc.tensor.matmul(
                                po,
                                wo_t[:, h, kd, dc * 128:(dc + 1) * 128],
                                attnT_u[:, kd, qb * 512:(qb + 1) * 512],
                                start=(kd == 0), stop=(kd == DC - 1),
                            )
                        nc.vector.tensor_copy(
                            out=o_t[:, dc, qb * 512:(qb + 1) * 512], in_=po)
                nc.sync.dma_start(out=out_d[h], in_=o_t)
            for p_ in reversed(phase_d_pools):
                p_.__exit__(None, None, None)

    return nc


_PROG_CACHE = {}


def _get_program() -> bass.Bass:
    if "nc" not in _PROG_CACHE:
        nc = build_program()
        nc.finalize()   # Bacc.compile(): wait splitting, reg alloc, act tables
        _PROG_CACHE["nc"] = nc
    return _PROG_CACHE["nc"]


def _host_prep(input_query, key, value, gq, bq_ln, gk, bk_ln, gv, bv_ln,
               Wq, bq, Wk, bk, Wv, bv, Wo, bo):
    """Fold LN affine + biases into weights; build per-core input maps."""
    bf = ml_dtypes.bfloat16
    scale = np.float32(1.0 / np.sqrt(KD))
    Wq_f = Wq.reshape(D, H * KD).astype(np.float32)
    Wk_f = Wk.reshape(D, H * KD).astype(np.float32)
    Wv_f = Wv.reshape(D, H * KD).astype(np.float32)
    bq_f = bq.reshape(H * KD).astype(np.float32)
    bv_f = bv.reshape(H * KD).astype(np.float32)
    # v-bias flows through softmax (rows sum to 1) -> constant through Wo.
    # k-bias is softmax-invariant (adds a per-q constant to scores) -> dropped.
    bv_eff = bv_ln.astype(np.float32) @ Wv_f + bv_f
    const_full = sum(
        bv_eff[h * KD:(h + 1) * KD] @ Wo[h].astype(np.float32) for h in range(H)
    ) + bo.astype(np.float32)  # [D]

    def chunked(w_eff):  # [D, HPC*KD] -> [128, DC, HPC*KD]
        return np.ascontiguousarray(
            w_eff.reshape(DC, 128, HPC * KD).transpose(1, 0, 2)
        )

    in_maps = []
    for c in range(NCORES):
        b, hg = c // 2, c % 2
        hsl = slice(hg * HPC * KD, (hg + 1) * HPC * KD)
        wq_eff = chunked(((gq[:, None] * Wq_f[:, hsl]) * scale).astype(bf))
        wk_eff = chunked((gk[:, None] * Wk_f[:, hsl]).astype(bf))
        wv_eff = chunked((gv[:, None] * Wv_f[:, hsl]).astype(bf))
        bq_eff = ((bq_ln.astype(np.float32) @ Wq_f[:, hsl] + bq_f[hsl]) * scale)
        bqt_np = np.ascontiguousarray(
            bq_eff.reshape(M_HKD, 128).T.astype(np.float32))          # [128, 8]
        # Wo slice: [128, HPC, DC, D]; [p,h,kd,d] = Wo[hg*4+h][kd*128+p, d]
        wo_np = np.ascontiguousarray(
            Wo[hg * HPC:(hg + 1) * HPC].astype(bf)
            .reshape(HPC, DC, 128, D).transpose(2, 0, 1, 3))
        in_maps.append({
            "xq": np.ascontiguousarray(input_query[b], np.float32),
            "xk": np.ascontiguousarray(key[b], np.float32),
            "xv": np.ascontiguousarray(value[b], np.float32),
            "wq": wq_eff, "wk": wk_eff, "wv": wv_eff,
            "wo": wo_np, "bqt": bqt_np,
        })
    return in_maps, const_full


def kernel(_trace=False, **inputs):
    inputs = {k: np.asarray(v) for k, v in inputs.items()}
    in_maps, const_full = _host_prep(**inputs)
    nc = _get_program()
    res = run_bass_kernel_spmd(nc, in_maps, core_ids=list(range(NCORES)),
                               trace=_trace)
    x_q = inputs["input_query"].astype(np.float32)
    out = np.empty((B, S, D), np.float32)
    for b in range(B):
        acc = np.zeros((S, D), np.float32)
        for hg in range(2):
            r = res.results[2 * b + hg]
            pT = r["outT"]                       # [HPC, 128, DC, S] unnormalized
            dens = r["dens"]                     # [HPC*N_S, 512]
            for h in range(HPC):
                mat = pT[h].transpose(1, 0, 2).reshape(D, S)     # [D, S]
                den = dens[h * N_S:(h + 1) * N_S].reshape(S)     # [S]
                acc += (mat / den[None, :]).T
        out[b] = x_q[b] + const_full[None, :] + acc
    if _trace:
        return out, res
    return out



# revision 8
# speedup vs baseline: 1.4082x; 1.4082x over previous
"""Fused pre-LN MHA kernel for Trainium2 (8 NeuronCores) — fp8 DoubleRow, v3.

v3 over v2: software-pipelined attention (block N scores/exp interleaved with
block N-1 den/PV/normalize at kb-pair granularity), q-stream woven into the
attention scope (no PSUM pool-swap barrier before attention), greedy engine
balancer pre-charged with DVE's phase-D mandatory load.

See kernel_v2 docstring for the full dataflow description.
"""

import numpy as np
import ml_dtypes

import concourse.bass as bass
import concourse.bacc as bacc
import concourse.mybir as mybir
import concourse.tile as tile
from concourse.bass_utils import run_bass_kernel_spmd
from concourse.masks import make_identity

B, S, D, H, KD = 4, 2048, 256, 8, 256
HPC = 4
NCORES = 8
EPS = 1e-5
FP = mybir.dt.float32
F32R = mybir.dt.float32r
BF = mybir.dt.bfloat16
F8 = mybir.dt.float8e4
F85 = mybir.dt.float8e5
I8 = mybir.dt.int8
AF = mybir.ActivationFunctionType
ALU = mybir.AluOpType
DR = mybir.MatmulPerfMode.DoubleRow

KB = S // 128
NQ = S // 512
XT = S // 128

LOG2E = 1.4426950408889634
SLOPE = 4.0 * LOG2E / 16.0
SBIAS = 44.0 + 0.25
SCHRAUD = True


def build_program() -> bass.Bass:
    nc = bacc.Bacc(None)

    xq = nc.declare_dram_parameter("xq", [S, D], FP, isOutput=False)
    xk = nc.declare_dram_parameter("xk", [S, D], FP, isOutput=False)
    xv = nc.declare_dram_parameter("xv", [S, D], FP, isOutput=False)
    wq = nc.declare_dram_parameter("wq", [128, 2, HPC * KD], F8, isOutput=False)
    wk = nc.declare_dram_parameter("wk", [128, 2, HPC * KD], F8, isOutput=False)
    wv = nc.declare_dram_parameter("wv", [128, 2, HPC * KD], F8, isOutput=False)
    wo = nc.declare_dram_parameter("wo", [128, 2, HPC, D], BF, isOutput=False)
    bqt = nc.declare_dram_parameter("bqt", [128, 2 * HPC], FP, isOutput=False)
    out_d = nc.declare_dram_parameter("out", [NQ, 128, 2, 512], BF,
                                      isOutput=True)
    out_re = out_d

    # greedy engine balancer (phase-local: reset at the attention scope)
    busy = {"A": 0.0, "D": 0.0, "P": 0.0}

    def pick(costs):
        e = min(costs, key=lambda k: busy[k] + costs[k])
        busy[e] += costs[e]
        return e

    with tile.TileContext(nc) as tc:
        with (
            tc.tile_pool(name="consts", bufs=1) as consts,
            tc.tile_pool(name="persist", bufs=1) as persist,
            tc.tile_pool(name="probsA", bufs=2) as probsA_p,
            tc.tile_pool(name="probsB", bufs=2) as probsB_p,
            tc.tile_pool(name="attn_n", bufs=2) as attn_p,
            tc.tile_pool(name="rden1", bufs=2) as rden1_p,
            tc.tile_pool(name="poS", bufs=2) as poS_p,
            tc.tile_pool(name="ostage", bufs=2) as ostage_p,
            tc.tile_pool(name="xraw", bufs=2) as xraw_p,
            tc.tile_pool(name="ln_small", bufs=2) as ln_small,
            tc.tile_pool(name="xhat", bufs=4) as xhat_p,
            tc.tile_pool(name="xhatT", bufs=2) as xhatT_p,
        ):
            # ---- constants ----
            ident = consts.tile([128, 128], BF)
            make_identity(nc, ident)
            ones8 = consts.tile([128, 2, 128], F8)
            nc.vector.memset(ones8, 1.0)
            eps_t = consts.tile([128, 1], FP)
            nc.vector.memset(eps_t, EPS)
            nln16_t = consts.tile([128, 1], FP)
            nc.vector.memset(nln16_t, -2.772588722239781)

            wq_t = consts.tile([128, 2, HPC * KD], F8)
            wk_t = consts.tile([128, 2, HPC * KD], F8)
            wv_t = consts.tile([128, 2, HPC * KD], F8)
            wo_t = consts.tile([128, 2, HPC, D], BF)
            bqt_t = consts.tile([128, 2 * HPC], FP)

            def load_weights():
                nc.sync.dma_start(out=wq_t, in_=wq[:])
                nc.sync.dma_start(out=wk_t, in_=wk[:])
                nc.sync.dma_start(out=wv_t, in_=wv[:])
                nc.sync.dma_start(out=wo_t, in_=wo[:])
                nc.sync.dma_start(out=bqt_t, in_=bqt[:])

            qT = persist.tile([128, HPC, 2, S], F8, tag="qT")
            kT = persist.tile([128, HPC, 2, S], F8, tag="kT")
            v8 = persist.tile([128, KB, HPC * KD], F8, tag="v8")

            # ---- shared elementwise helpers (greedy engine choice) ----
            def copy_scaled(dst, ps, bias=None):
                # gpsimd cannot read PSUM on HW
                e = pick({"A": 612, "D": 593})
                if e == "A":
                    nc.scalar.activation(
                        out=dst, in_=ps, func=AF.Identity, scale=1.0 / 16.0,
                        **({"bias": bias} if bias is not None else {}))
                else:
                    eng = nc.vector if e == "D" else nc.gpsimd
                    if bias is None:
                        eng.tensor_scalar(out=dst, in0=ps, scalar1=1.0 / 16.0,
                                          scalar2=None, op0=ALU.mult)
                    else:
                        eng.tensor_scalar(out=dst, in0=ps, scalar1=1.0 / 16.0,
                                          scalar2=bias, op0=ALU.mult,
                                          op1=ALU.add)

            def tr_copy(dst, ps):
                e = pick({"A": 292, "D": 194})
                if e == "A":
                    nc.scalar.copy(out=dst, in_=ps)
                else:
                    (nc.vector if e == "D" else nc.gpsimd).tensor_copy(
                        out=dst, in_=ps)

            def ln_group(xr, mv, r, g, xhatT, tr_pool, first=False):
                """LN for tiles g*4..g*4+3 + transposes into xhatT."""
                for i in range(g * 4, g * 4 + 4):
                    stats = ln_small.tile([128, 6], FP, tag="stats")
                    nc.vector.bn_stats(out=stats, in_=xr[:, i])
                    nc.vector.bn_aggr(out=mv[:, i], in_=stats)
                sl = slice(g * 4, g * 4 + 4)
                sd = ln_small.tile([128, 4], FP, tag="sd")
                nc.scalar.activation(out=sd, in_=mv[:, sl, 1], func=AF.Sqrt,
                                     bias=eps_t[:], scale=1.0)
                r_sl = r[:, sl]
                nc.vector.reciprocal(out=r_sl, in_=sd)
                busy["D"] += 1400
                for i in range(g * 4, g * 4 + 4):
                    xh = xhat_p.tile([128, D], BF, tag="xh")
                    e = pick({"A": 398, "D": 327, "P": 603})
                    if e in ("D", "P"):
                        (nc.vector if e == "D" else nc.gpsimd).tensor_scalar(
                            out=xh, in0=xr[:, i], scalar1=mv[:, i, 0:1],
                            scalar2=r[:, i:i + 1],
                            op0=ALU.subtract, op1=ALU.mult)
                    else:
                        negmr = ln_small.tile([128, 1], FP, tag="negmr")
                        nc.vector.tensor_scalar(
                            out=negmr, in0=mv[:, i, 0:1],
                            scalar1=r[:, i:i + 1], scalar2=-1.0,
                            op0=ALU.mult, op1=ALU.mult)
                        busy["D"] += 67
                        nc.scalar.activation(
                            out=xh, in_=xr[:, i], func=AF.Identity,
                            bias=negmr[:], scale=r[:, i:i + 1])
                    for c in range(2):
                        pst = tr_pool.tile([128, 128], BF, name="pst",
                                           tag=tr_pool.name_tag)
                        nc.tensor.transpose(
                            pst, xh[:, c * 128:(c + 1) * 128], ident)
                        tr_copy(xhatT[:, c, i * 128:(i + 1) * 128], pst)

            class PoolView:
                def __init__(self, pool, tag):
                    self.pool = pool
                    self.name_tag = tag

                def tile(self, shape, dt, name=None, tag=None):
                    return self.pool.tile(shape, dt, name=name,
                                          tag=tag or self.name_tag)

            def proj_chunk(dst, w_t, xhatT, nb, biased, ps_pool, tag):
                for m in range(2 * HPC):
                    h, c = divmod(m, 2)
                    ps = ps_pool.tile([128, 512], FP, name="psp", tag=tag)
                    nc.tensor.matmul(
                        ps, w_t[:, :, m * 128:(m + 1) * 128],
                        xhatT[:, :, nb * 512:(nb + 1) * 512],
                        start=True, stop=True, perf_mode=DR)
                    copy_scaled(dst[:, h, c, nb * 512:(nb + 1) * 512], ps,
                                bias=bqt_t[:, m:m + 1] if biased else None)

            # ======== scope 1: k and v streams ========
            with tc.tile_pool(name="ps_ac", bufs=6,
                              space=bass.MemorySpace.PSUM) as ps_ac:
                pv_ac = PoolView(ps_ac, "ac")
                xhatT_k = xhatT_p.tile([128, 2, S], F8, name="xhT_k", tag="xhT")
                xre_k = xk.rearrange("(c t p) d -> c p t d", t=4, p=128)
                xr_k = xraw_p.tile([128, XT, D], FP, tag="xraw")
                mv_k = ln_small.tile([128, XT, 2], FP, tag="mv")
                r_k = ln_small.tile([128, XT], FP, tag="r")
                for g in range(4):
                    nc.sync.dma_start(out=xr_k[:, g * 4:(g + 1) * 4],
                                      in_=xre_k[g])
                    if g == 0:
                        load_weights()
                    ln_group(xr_k, mv_k, r_k, g, xhatT_k, pv_ac)
                for nb in range(NQ):
                    proj_chunk(kT, wk_t, xhatT_k, nb, False, ps_ac, "ac")
                xhatT_v = xhatT_p.tile([128, 2, S], F8, name="xhT_v", tag="xhT")
                xre_v = xv.rearrange("(c t p) d -> c p t d", t=4, p=128)
                xr_v = xraw_p.tile([128, XT, D], FP, tag="xraw")
                mv_v = ln_small.tile([128, XT, 2], FP, tag="mv")
                r_v = ln_small.tile([128, XT], FP, tag="r")
                for g in range(4):
                    nc.sync.dma_start(out=xr_v[:, g * 4:(g + 1) * 4],
                                      in_=xre_v[g])
                    ln_group(xr_v, mv_v, r_v, g, xhatT_v, pv_ac)
                    for i2 in range(g * 4, g * 4 + 4):
                        for n in range(2):
                            ps = ps_ac.tile([128, 512], FP, name="psp",
                                            tag="ac")
                            nc.tensor.matmul(
                                ps, xhatT_v[:, :, i2 * 128:(i2 + 1) * 128],
                                wv_t[:, :, n * 512:(n + 1) * 512],
                                start=True, stop=True, perf_mode=DR)
                            copy_scaled(v8[:, i2, n * 512:(n + 1) * 512], ps)

            # ======== scope 2: q stream woven with pipelined attention ====
            phase_d_pools = (
                tc.tile_pool(name="ps_s", bufs=4, space=bass.MemorySpace.PSUM),
                tc.tile_pool(name="ps_pv", bufs=1, space=bass.MemorySpace.PSUM),
                tc.tile_pool(name="ps_den", bufs=1, space=bass.MemorySpace.PSUM),
                tc.tile_pool(name="ps_shared", bufs=1,
                             space=bass.MemorySpace.PSUM),
            )
            busy.update({"A": 0.0, "D": 0.0, "P": 0.0})
            ps_s = phase_d_pools[0].__enter__()
            ps_pv = phase_d_pools[1].__enter__()
            ps_den = phase_d_pools[2].__enter__()
            ps_sh = phase_d_pools[3].__enter__()
            pv_ss = PoolView(ps_s, "ss")

            # --- q stream emission helpers (woven below) ---
            xhatT_q = xhatT_p.tile([128, 2, S], F8, name="xhT_q", tag="xhT")
            xre_q = xq.rearrange("(c t p) d -> c p t d", t=4, p=128)
            xr_q = xraw_p.tile([128, XT, D], FP, tag="xraw")
            mv_q = ln_small.tile([128, XT, 2], FP, tag="mv")
            r_q = ln_small.tile([128, XT], FP, tag="r")

            def q_chunk(g):
                # g in 0..1: two 4-tile LN groups + one 512-col projection nb
                for gg in (2 * g, 2 * g + 1):
                    nc.sync.dma_start(out=xr_q[:, gg * 4:(gg + 1) * 4],
                                      in_=xre_q[gg])
                    ln_group(xr_q, mv_q, r_q, gg, xhatT_q, pv_ss)
                for nb in (2 * g, 2 * g + 1):
                    proj_chunk(qT, wq_t, xhatT_q, nb, True, ps_s, "ss")

            # --- attention block pieces ---
            attnN = {}

            def front(qb, h):
                pA = probsA_p.tile([128, 16, 512], F8, tag="pA")
                pB = probsB_p.tile([128, 16, 512], F85, tag="pB")
                peng = []
                na = nb_ = 0
                for j in range(8):
                    e = pick({"A": 1224, "D": 1186}) if SCHRAUD else pick({"A": 1224})
                    if e == "A":
                        peng.append(("A", na)); na += 1
                    else:
                        peng.append((e, nb_)); nb_ += 1
                prs = []
                emits = []

                def emit_pair(j):
                    e, slot = peng[j]
                    for half in range(2):
                        kb = 2 * j + half
                        ss = ps_s.tile([128, 512], FP, tag="ss")
                        nc.tensor.matmul(
                            ss, kT[:, h, :, kb * 128:(kb + 1) * 128],
                            qT[:, h, :, qb * 512:(qb + 1) * 512],
                            start=True, stop=True, perf_mode=DR)
                        if e == "A":
                            nc.scalar.activation(
                                out=pA[:, 2 * slot + half], in_=ss,
                                func=AF.Exp, bias=nln16_t[:], scale=1.0 / 16.0)
                        else:
                            nc.vector.tensor_scalar(
                                out=pB[:, 2 * slot + half].bitcast(I8),
                                in0=ss, scalar1=SLOPE, scalar2=SBIAS,
                                op0=ALU.mult, op1=ALU.add)
                    src = pA if e == "A" else pB
                    prs.append(src[:, 2 * slot:2 * slot + 2])
                return {"qb": qb, "h": h, "prs": prs, "emit_pair": emit_pair,
                        "pv": None, "den": None}

            def tail_den(st, j):
                if j == 0:
                    st["den"] = ps_den.tile([128, 512], FP, name="den", tag="den")
                nc.tensor.matmul(st["den"], ones8, st["prs"][j],
                                 start=(j == 0), stop=(j == 7), perf_mode=DR)

            def tail_pv(st, j):
                if j == 0:
                    st["pv"] = ps_pv.tile([128, 2, 512], FP, name="pv", tag="pv")
                h = st["h"]
                for m in range(2):
                    nc.tensor.matmul(
                        st["pv"][:, m],
                        v8[:, 2 * j:2 * j + 2,
                           h * KD + m * 128:h * KD + (m + 1) * 128],
                        st["prs"][j],
                        start=(j == 0), stop=(j == 7), perf_mode=DR)

            def tail_recip(st):
                rcp = rden1_p.tile([128, 512], FP, tag="rd1")
                nc.vector.reciprocal(out=rcp, in_=st["den"])
                busy["D"] += 2100
                st["rcp"] = rcp

            def tail_norm(st):
                qb, h = st["qb"], st["h"]
                for m in range(2):
                    nc.vector.tensor_tensor(
                        out=attnN[qb][:, h, m], in0=st["pv"][:, m],
                        in1=st["rcp"], op=ALU.mult)

            def epilogue(qb):
                poS = poS_p.tile([128, 2, 512], BF, tag="poS")
                for dc in range(2):
                    po = ps_sh.tile([128, 512], FP, tag="sh")
                    for hh in range(HPC):
                        for c in range(2):
                            nc.tensor.matmul(
                                po, wo_t[:, c, hh, dc * 128:(dc + 1) * 128],
                                attnN[qb][:, hh, c],
                                start=(hh == 0 and c == 0),
                                stop=(hh == HPC - 1 and c == 1))
                    e = pick({"A": 612, "D": 593})
                    if e == "A":
                        nc.scalar.copy(out=poS[:, dc], in_=po)
                    else:
                        nc.vector.tensor_copy(out=poS[:, dc], in_=po)
                nc.sync.dma_start(out=out_re[qb], in_=poS)

            # --- woven schedule ---
            blocks = [(qb, h) for qb in range(NQ) for h in range(HPC)]
            prev = None
            for idx, (qb, h) in enumerate(blocks):
                if idx == 0:
                    q_chunk(0)
                if idx == 1:
                    q_chunk(1)
                if h == 0:
                    attnN[qb] = attn_p.tile([128, HPC, 2, 512], BF,
                                            name="attnN", tag="attnN")
                st = front(qb, h)
                for j in range(8):
                    st["emit_pair"](j)
                    if prev is not None:
                        if j < 4:
                            tail_den(prev, 2 * j)
                            tail_den(prev, 2 * j + 1)
                        tail_pv(prev, j)
                        if j == 4:
                            tail_recip(prev)
                if prev is not None:
                    tail_norm(prev)
                    if prev["h"] == HPC - 1:
                        epilogue(prev["qb"])
                prev = st
            for j in range(8):
                tail_den(prev, j)
                tail_pv(prev, j)
            tail_recip(prev)
            tail_norm(prev)
            epilogue(prev["qb"])
            for p_ in reversed(phase_d_pools):
                p_.__exit__(None, None, None)

    return nc


_PROG_CACHE = {}


def _get_program() -> bass.Bass:
    if "nc" not in _PROG_CACHE:
        nc = build_program()
        nc.finalize()
        _PROG_CACHE["nc"] = nc
    return _PROG_CACHE["nc"]


def _host_prep(input_query, key, value, gq, bq_ln, gk, bk_ln, gv, bv_ln,
               Wq, bq, Wk, bk, Wv, bv, Wo, bo):
    f8 = ml_dtypes.float8_e4m3
    bf = ml_dtypes.bfloat16
    Wq_f = Wq.reshape(D, H * KD).astype(np.float32)
    Wk_f = Wk.reshape(D, H * KD).astype(np.float32)
    Wv_f = Wv.reshape(D, H * KD).astype(np.float32)
    bq_f = bq.reshape(H * KD).astype(np.float32)
    bv_f = bv.reshape(H * KD).astype(np.float32)
    bv_eff = bv_ln.astype(np.float32) @ Wv_f + bv_f
    const_full = sum(
        bv_eff[h * KD:(h + 1) * KD] @ Wo[h].astype(np.float32) for h in range(H)
    ) + bo.astype(np.float32)

    def chunked8(w_eff):
        return np.ascontiguousarray(
            (16.0 * w_eff).reshape(2, 128, HPC * KD).transpose(1, 0, 2)
        ).astype(f8)

    in_maps = []
    for c in range(NCORES):
        b, hg = c // 2, c % 2
        hsl = slice(hg * HPC * KD, (hg + 1) * HPC * KD)
        wq8 = chunked8(gq[:, None] * Wq_f[:, hsl])
        wk8 = chunked8(gk[:, None] * Wk_f[:, hsl])
        wv8 = chunked8(gv[:, None] * Wv_f[:, hsl])
        bq_eff = bq_ln.astype(np.float32) @ Wq_f[:, hsl] + bq_f[hsl]
        bqt_np = np.ascontiguousarray(
            bq_eff.reshape(2 * HPC, 128).T.astype(np.float32))
        wo_np = np.ascontiguousarray(
            Wo[hg * HPC:(hg + 1) * HPC].astype(np.float32)
            .reshape(HPC, 2, 128, D).transpose(2, 1, 0, 3)).astype(bf)
        in_maps.append({
            "xq": np.ascontiguousarray(input_query[b], np.float32),
            "xk": np.ascontiguousarray(key[b], np.float32),
            "xv": np.ascontiguousarray(value[b], np.float32),
            "wq": wq8, "wk": wk8, "wv": wv8,
            "wo": wo_np, "bqt": bqt_np,
        })
    return in_maps, const_full


def kernel(_trace=False, **inputs):
    inputs = {k: np.asarray(v) for k, v in inputs.items()}
    in_maps, const_full = _host_prep(**inputs)
    nc = _get_program()
    res = run_bass_kernel_spmd(nc, in_maps, core_ids=list(range(NCORES)),
                               trace=_trace)
    x_q = inputs["input_query"].astype(np.float32)
    out = np.empty((B, S, D), np.float32)
    for b in range(B):
        # device out: [NQ, 128(p=do%128), 2(dc), 512(col)] ->
        #   out[qb*512+col, dc*128+p]
        a0 = res.results[2 * b]["out"].astype(np.float32)
        a1 = res.results[2 * b + 1]["out"].astype(np.float32)
        part = (a0 + a1).transpose(0, 3, 2, 1).reshape(S, D)
        out[b] = x_q[b] + const_full[None, :] + part
    if _trace:
        return out, res
    return out


# revision 12
# speedup vs baseline: 2.2962x; 1.6305x over previous
"""Fused pre-LN multi-head attention kernel for Trainium2 (8 NeuronCores).

Problem: B=4, S=2048, D=256, H=8, KD=256.
    out = x_q + MHA(LN(x_q), LN(x_k), LN(x_v))   (keras-style, q scaled 1/sqrt(KD))

Sharding: core c -> batch b = c//2, head group hg = c%2 (4 heads each).

Device pipeline (per core), built around fp8e4 DoubleRow matmuls (K=256 per
instruction via [128, 2, *] access patterns, 0.5 PE cycles per output row):

  A) k-stream LayerNorm (bn_stats/bn_aggr on DVE, batched ACT sqrt), xhat in
     bf16, PE-transpose -> xhatT [d, s], convert to fp8 in the PSUM->SBUF
     copy.  k projection via DoubleRow; weights are host-prescaled x16 and
     the copy applies 1/16 (+ folded q bias), landing q/k/v at true scale
     in fp8e4.
  B) v and q streams are emitted inside the attention scope so their LN /
     projection work overlaps the first attention blocks (scores only need
     kT + qT(block 0); PV consumes v just in time).
  C) Attention, software-pipelined at kb-pair granularity: block N's
     16 scores matmuls + exps interleave with block N-1's den/PV matmuls.
     exp is split across engines per pair (greedy load balancer):
       ACT: exact exp -> fp8e4 probs  (scale=1/16, bias=-ln16 folded in)
       DVE: Schraudolph exp -> int8 bitcast = fp8e5 probs
     den uses an all-ones [128,2,128] fp8 lhsT so one DoubleRow accumulation
     yields the denominator broadcast across all 128 partitions; reciprocal
     (DVE) + one broadcast multiply normalize PV -> bf16 attnN.  The output
     projection runs in bf16 (precision headroom) accumulating all 4 heads
     in one PSUM bank per 128-wide d chunk.
  D) Output is DMAed transposed ([do, q] bf16, 1MB/core); the host
     untransposes, adds the residual x_q and folded bias constants, and sums
     the two head-group cores per batch.

Known HW constraints honored: gpsimd cannot touch PSUM (it only gets the
SBUF-only xhat work); tensor ops read at most one PSUM operand; fp8
transpose needs stride-2 output (so transposes run in bf16); matmul outputs
are fp32 (TRN2).

Numerics (validated vs the jax reference, seed 0): rel err ~1.31e-2 of
which fp8 quantization of the scores path dominates; threshold 2e-2.
"""

import numpy as np
import ml_dtypes

import concourse.bass as bass
import concourse.bacc as bacc
import concourse.mybir as mybir
import concourse.tile as tile
from concourse.bass_utils import run_bass_kernel_spmd
from concourse.masks import make_identity

B, S, D, H, KD = 4, 2048, 256, 8, 256
HPC = 4
NCORES = 8
EPS = 1e-5
FP = mybir.dt.float32
F32R = mybir.dt.float32r
BF = mybir.dt.bfloat16
F8 = mybir.dt.float8e4
F85 = mybir.dt.float8e5
I8 = mybir.dt.int8
AF = mybir.ActivationFunctionType
ALU = mybir.AluOpType
DR = mybir.MatmulPerfMode.DoubleRow

KB = S // 128
NQ = S // 512
XT = S // 128

LOG2E = 1.4426950408889634
SLOPE = 4.0 * LOG2E / 16.0
SBIAS = 44.0 + 0.25
SCHRAUD = True


def build_program() -> bass.Bass:
    nc = bacc.Bacc(None)

    xq = nc.declare_dram_parameter("xq", [S, D], FP, isOutput=False)
    xk = nc.declare_dram_parameter("xk", [S, D], FP, isOutput=False)
    xv = nc.declare_dram_parameter("xv", [S, D], FP, isOutput=False)
    wq = nc.declare_dram_parameter("wq", [128, 2, HPC * KD], F8, isOutput=False)
    wk = nc.declare_dram_parameter("wk", [128, 2, HPC * KD], F8, isOutput=False)
    wv = nc.declare_dram_parameter("wv", [128, 2, HPC * KD], F8, isOutput=False)
    wo = nc.declare_dram_parameter("wo", [128, 2, HPC, D], BF, isOutput=False)
    bqt = nc.declare_dram_parameter("bqt", [128, 2 * HPC], FP, isOutput=False)
    out_d = nc.declare_dram_parameter("out", [NQ, 128, 2, 512], BF,
                                      isOutput=True)
    out_re = out_d

    # greedy engine balancer (phase-local: reset at the attention scope)
    busy = {"A": 0.0, "D": 0.0, "P": 0.0}

    def pick(costs):
        e = min(costs, key=lambda k: busy[k] + costs[k])
        busy[e] += costs[e]
        return e

    with tile.TileContext(nc) as tc:
        with (
            tc.tile_pool(name="consts", bufs=1) as consts,
            tc.tile_pool(name="persist", bufs=1) as persist,
            tc.tile_pool(name="probsA", bufs=2) as probsA_p,
            tc.tile_pool(name="probsB", bufs=2) as probsB_p,
            tc.tile_pool(name="attn_n", bufs=2) as attn_p,
            tc.tile_pool(name="rden1", bufs=2) as rden1_p,
            tc.tile_pool(name="poS", bufs=2) as poS_p,
            tc.tile_pool(name="ostage", bufs=2) as ostage_p,
            tc.tile_pool(name="xraw", bufs=2) as xraw_p,
            tc.tile_pool(name="ln_small", bufs=2) as ln_small,
            tc.tile_pool(name="xhat", bufs=4) as xhat_p,
            tc.tile_pool(name="xhatT", bufs=2) as xhatT_p,
        ):
            # ---- constants ----
            ident = consts.tile([128, 128], BF)
            make_identity(nc, ident)
            ones8 = consts.tile([128, 2, 128], F8)
            nc.vector.memset(ones8, 1.0)
            eps_t = consts.tile([128, 1], FP)
            nc.vector.memset(eps_t, EPS)
            nln16_t = consts.tile([128, 1], FP)
            nc.vector.memset(nln16_t, -2.772588722239781)

            wq_t = consts.tile([128, 2, HPC * KD], F8)
            wk_t = consts.tile([128, 2, HPC * KD], F8)
            wv_t = consts.tile([128, 2, HPC * KD], F8)
            wo_t = consts.tile([128, 2, HPC, D], BF)
            bqt_t = consts.tile([128, 2 * HPC], FP)

            def load_weights():
                nc.sync.dma_start(out=wq_t, in_=wq[:])
                nc.sync.dma_start(out=wk_t, in_=wk[:])
                nc.sync.dma_start(out=wv_t, in_=wv[:])
                nc.sync.dma_start(out=wo_t, in_=wo[:])
                nc.sync.dma_start(out=bqt_t, in_=bqt[:])

            qT = persist.tile([128, HPC, 2, S], F8, tag="qT")
            kT = persist.tile([128, HPC, 2, S], F8, tag="kT")
            v8 = persist.tile([128, KB, HPC * KD], F8, tag="v8")

            # ---- shared elementwise helpers (greedy engine choice) ----
            def copy_scaled(dst, ps, bias=None):
                # gpsimd cannot read PSUM on HW
                e = pick({"A": 612, "D": 593})
                if e == "A":
                    nc.scalar.activation(
                        out=dst, in_=ps, func=AF.Identity, scale=1.0 / 16.0,
                        **({"bias": bias} if bias is not None else {}))
                else:
                    eng = nc.vector if e == "D" else nc.gpsimd
                    if bias is None:
                        eng.tensor_scalar(out=dst, in0=ps, scalar1=1.0 / 16.0,
                                          scalar2=None, op0=ALU.mult)
                    else:
                        eng.tensor_scalar(out=dst, in0=ps, scalar1=1.0 / 16.0,
                                          scalar2=bias, op0=ALU.mult,
                                          op1=ALU.add)

            def tr_copy(dst, ps):
                e = pick({"A": 398, "D": 327})
                if e == "A":
                    nc.scalar.copy(out=dst, in_=ps)
                else:
                    (nc.vector if e == "D" else nc.gpsimd).tensor_copy(
                        out=dst, in_=ps)

            def ln_group(xr, mv, r, g, xhatT, tr_pool, first=False):
                """LN for tiles g*4..g*4+3 + transposes into xhatT."""
                for i in range(g * 4, g * 4 + 4):
                    stats = ln_small.tile([128, 6], FP, tag="stats")
                    nc.vector.bn_stats(out=stats, in_=xr[:, i])
                    nc.vector.bn_aggr(out=mv[:, i], in_=stats)
                sl = slice(g * 4, g * 4 + 4)
                sd = ln_small.tile([128, 4], FP, tag="sd")
                nc.scalar.activation(out=sd, in_=mv[:, sl, 1], func=AF.Sqrt,
                                     bias=eps_t[:], scale=1.0)
                r_sl = r[:, sl]
                nc.vector.reciprocal(out=r_sl, in_=sd)
                busy["D"] += 1400
                for i in range(g * 4, g * 4 + 4):
                    xh = xhat_p.tile([128, D], BF, tag="xh")
                    e = pick({"A": 398, "D": 327, "P": 603})
                    if e in ("D", "P"):
                        (nc.vector if e == "D" else nc.gpsimd).tensor_scalar(
                            out=xh, in0=xr[:, i], scalar1=mv[:, i, 0:1],
                            scalar2=r[:, i:i + 1],
                            op0=ALU.subtract, op1=ALU.mult)
                    else:
                        negmr = ln_small.tile([128, 1], FP, tag="negmr")
                        nc.vector.tensor_scalar(
                            out=negmr, in0=mv[:, i, 0:1],
                            scalar1=r[:, i:i + 1], scalar2=-1.0,
                            op0=ALU.mult, op1=ALU.mult)
                        busy["D"] += 67
                        nc.scalar.activation(
                            out=xh, in_=xr[:, i], func=AF.Identity,
                            bias=negmr[:], scale=r[:, i:i + 1])
                    pst = tr_pool.tile([128, 2, 128], BF, name="pst",
                                       tag=tr_pool.name_tag)
                    for c in range(2):
                        nc.tensor.transpose(
                            pst[:, c], xh[:, c * 128:(c + 1) * 128], ident)
                    tr_copy(xhatT[:, :, i * 128:(i + 1) * 128], pst)

            class PoolView:
                def __init__(self, pool, tag):
                    self.pool = pool
                    self.name_tag = tag

                def tile(self, shape, dt, name=None, tag=None):
                    return self.pool.tile(shape, dt, name=name,
                                          tag=tag or self.name_tag)

            def proj_chunk(dst, w_t, xhatT, nb, biased, ps_pool, tag):
                for m in range(2 * HPC):
                    h, c = divmod(m, 2)
                    ps = ps_pool.tile([128, 512], FP, name="psp", tag=tag)
                    nc.tensor.matmul(
                        ps, w_t[:, :, m * 128:(m + 1) * 128],
                        xhatT[:, :, nb * 512:(nb + 1) * 512],
                        start=True, stop=True, perf_mode=DR)
                    copy_scaled(dst[:, h, c, nb * 512:(nb + 1) * 512], ps,
                                bias=bqt_t[:, m:m + 1] if biased else None)

            # ======== scope 1: k stream only ========
            with tc.tile_pool(name="ps_ac", bufs=6,
                              space=bass.MemorySpace.PSUM) as ps_ac:
                pv_ac = PoolView(ps_ac, "ac")
                xhatT_k = xhatT_p.tile([128, 2, S], F8, name="xhT_k", tag="xhT")
                xre_k = xk.rearrange("(c t p) d -> c p t d", t=4, p=128)
                xr_k = xraw_p.tile([128, XT, D], FP, tag="xraw")
                mv_k = ln_small.tile([128, XT, 2], FP, tag="mv")
                r_k = ln_small.tile([128, XT], FP, tag="r")
                for g in range(4):
                    nc.sync.dma_start(out=xr_k[:, g * 4:(g + 1) * 4],
                                      in_=xre_k[g])
                    if g == 0:
                        load_weights()
                    ln_group(xr_k, mv_k, r_k, g, xhatT_k, pv_ac)
                k_proj_pending = True

            # ======== scope 2: q stream woven with pipelined attention ====
            phase_d_pools = (
                tc.tile_pool(name="ps_s", bufs=4, space=bass.MemorySpace.PSUM),
                tc.tile_pool(name="ps_pv", bufs=1, space=bass.MemorySpace.PSUM),
                tc.tile_pool(name="ps_den", bufs=1, space=bass.MemorySpace.PSUM),
                tc.tile_pool(name="ps_shared", bufs=1,
                             space=bass.MemorySpace.PSUM),
            )
            busy.update({"A": 0.0, "D": 0.0, "P": 0.0})
            ps_s = phase_d_pools[0].__enter__()
            ps_pv = phase_d_pools[1].__enter__()
            ps_den = phase_d_pools[2].__enter__()
            ps_sh = phase_d_pools[3].__enter__()
            pv_ss = PoolView(ps_s, "ss")

            # --- q and v stream emission helpers (woven below) ---
            xhatT_q = xhatT_p.tile([128, 2, S], F8, name="xhT_q", tag="xhT")
            xre_q = xq.rearrange("(c t p) d -> c p t d", t=4, p=128)
            xr_q = xraw_p.tile([128, XT, D], FP, tag="xraw")
            mv_q = ln_small.tile([128, XT, 2], FP, tag="mv")
            r_q = ln_small.tile([128, XT], FP, tag="r")
            xhatT_v = xhatT_p.tile([128, 2, S], F8, name="xhT_v", tag="xhT")
            xre_v = xv.rearrange("(c t p) d -> c p t d", t=4, p=128)
            xr_v = xraw_p.tile([128, XT, D], FP, tag="xraw")
            mv_v = ln_small.tile([128, XT, 2], FP, tag="mv")
            r_v = ln_small.tile([128, XT], FP, tag="r")

            def q_chunk(g):
                # g in 0..1: two 4-tile LN groups + two 512-col projection nbs
                for gg in (2 * g, 2 * g + 1):
                    nc.sync.dma_start(out=xr_q[:, gg * 4:(gg + 1) * 4],
                                      in_=xre_q[gg])
                    ln_group(xr_q, mv_q, r_q, gg, xhatT_q, pv_ss)
                for nb in (2 * g, 2 * g + 1):
                    proj_chunk(qT, wq_t, xhatT_q, nb, True, ps_s, "ss")

            def v_chunk(g):
                # g in 0..1: two 4-tile LN groups + projections for kb g*8..+8
                for gg in (2 * g, 2 * g + 1):
                    nc.sync.dma_start(out=xr_v[:, gg * 4:(gg + 1) * 4],
                                      in_=xre_v[gg])
                    ln_group(xr_v, mv_v, r_v, gg, xhatT_v, pv_ss)
                for i2 in range(g * 8, g * 8 + 8):
                    for n in range(2):
                        ps = ps_s.tile([128, 512], FP, name="psp", tag="ss")
                        nc.tensor.matmul(
                            ps, xhatT_v[:, :, i2 * 128:(i2 + 1) * 128],
                            wv_t[:, :, n * 512:(n + 1) * 512],
                            start=True, stop=True, perf_mode=DR)
                        copy_scaled(v8[:, i2, n * 512:(n + 1) * 512], ps)

            # --- attention block pieces ---
            attnN = {}

            def front(qb, h):
                pA = probsA_p.tile([128, 16, 512], F8, tag="pA")
                pB = probsB_p.tile([128, 16, 512], F85, tag="pB")
                peng = []
                na = nb_ = 0
                for j in range(8):
                    e = pick({"A": 1224, "D": 1186}) if SCHRAUD else pick({"A": 1224})
                    if e == "A":
                        peng.append(("A", na)); na += 1
                    else:
                        peng.append((e, nb_)); nb_ += 1
                prs = []
                emits = []

                def emit_pair(j):
                    e, slot = peng[j]
                    for half in range(2):
                        kb = 2 * j + half
                        ss = ps_s.tile([128, 512], FP, name="ss", tag="ss")
                        nc.tensor.matmul(
                            ss, kT[:, h, :, kb * 128:(kb + 1) * 128],
                            qT[:, h, :, qb * 512:(qb + 1) * 512],
                            start=True, stop=True, perf_mode=DR)
                        if e == "A":
                            nc.scalar.activation(
                                out=pA[:, 2 * slot + half], in_=ss,
                                func=AF.Exp, bias=nln16_t[:], scale=1.0 / 16.0)
                        else:
                            nc.vector.tensor_scalar(
                                out=pB[:, 2 * slot + half].bitcast(I8),
                                in0=ss, scalar1=SLOPE, scalar2=SBIAS,
                                op0=ALU.mult, op1=ALU.add)
                    src = pA if e == "A" else pB
                    prs.append(src[:, 2 * slot:2 * slot + 2])
                return {"qb": qb, "h": h, "prs": prs, "emit_pair": emit_pair,
                        "pv": None, "den": None}

            def tail_den(st, j):
                if j == 0:
                    st["den"] = ps_den.tile([128, 512], FP, name="den", tag="den")
                nc.tensor.matmul(st["den"], ones8, st["prs"][j],
                                 start=(j == 0), stop=(j == 7), perf_mode=DR)

            def tail_pv(st, j):
                if j == 0:
                    st["pv"] = ps_pv.tile([128, 2, 512], FP, name="pv", tag="pv")
                h = st["h"]
                for m in range(2):
                    nc.tensor.matmul(
                        st["pv"][:, m],
                        v8[:, 2 * j:2 * j + 2,
                           h * KD + m * 128:h * KD + (m + 1) * 128],
                        st["prs"][j],
                        start=(j == 0), stop=(j == 7), perf_mode=DR)

            def tail_recip(st):
                rcp = rden1_p.tile([128, 512], FP, tag="rd1")
                nc.vector.reciprocal(out=rcp, in_=st["den"])
                busy["D"] += 2100
                st["rcp"] = rcp

            def tail_norm(st):
                qb, h = st["qb"], st["h"]
                rcp_b = st["rcp"].unsqueeze(1).broadcast_to([128, 2, 512])
                nc.vector.tensor_tensor(
                    out=attnN[qb][:, h], in0=st["pv"], in1=rcp_b,
                    op=ALU.mult)

            def epilogue(qb):
                poS = poS_p.tile([128, 2, 512], BF, tag="poS")
                for dc in range(2):
                    po = ps_sh.tile([128, 512], FP, tag="sh")
                    for hh in range(HPC):
                        for c in range(2):
                            nc.tensor.matmul(
                                po, wo_t[:, c, hh, dc * 128:(dc + 1) * 128],
                                attnN[qb][:, hh, c],
                                start=(hh == 0 and c == 0),
                                stop=(hh == HPC - 1 and c == 1))
                    e = pick({"A": 612, "D": 593})
                    if e == "A":
                        nc.scalar.copy(out=poS[:, dc], in_=po)
                    else:
                        nc.vector.tensor_copy(out=poS[:, dc], in_=po)
                nc.sync.dma_start(out=out_re[qb], in_=poS)

            # --- woven schedule ---
            blocks = [(qb, h) for qb in range(NQ) for h in range(HPC)]
            prev = None
            for nb in range(NQ):
                proj_chunk(kT, wk_t, xhatT_k, nb, False, ps_s, "ss")
            for idx, (qb, h) in enumerate(blocks):
                if idx == 0:
                    q_chunk(0)
                    v_chunk(0)
                if idx == 1:
                    v_chunk(1)
                if idx == 2:
                    q_chunk(1)
                if h == 0:
                    attnN[qb] = attn_p.tile([128, HPC, 2, 512], BF,
                                            name="attnN", tag="attnN")
                st = front(qb, h)
                for j in range(8):
                    st["emit_pair"](j)
                    if prev is not None:
                        if j < 4:
                            tail_den(prev, 2 * j)
                            tail_den(prev, 2 * j + 1)
                        tail_pv(prev, j)
                        if j == 4:
                            tail_recip(prev)
                if prev is not None:
                    tail_norm(prev)
                    if prev["h"] == HPC - 1:
                        epilogue(prev["qb"])
                prev = st
            for j in range(8):
                tail_den(prev, j)
                tail_pv(prev, j)
            tail_recip(prev)
            tail_norm(prev)
            epilogue(prev["qb"])
            for p_ in reversed(phase_d_pools):
                p_.__exit__(None, None, None)

    return nc


_PROG_CACHE = {}


def _get_program() -> bass.Bass:
    if "nc" not in _PROG_CACHE:
        nc = build_program()
        nc.finalize()
        _PROG_CACHE["nc"] = nc
    return _PROG_CACHE["nc"]


def _host_prep(input_query, key, value, gq, bq_ln, gk, bk_ln, gv, bv_ln,
               Wq, bq, Wk, bk, Wv, bv, Wo, bo):
    f8 = ml_dtypes.float8_e4m3
    bf = ml_dtypes.bfloat16
    Wq_f = Wq.reshape(D, H * KD).astype(np.float32)
    Wk_f = Wk.reshape(D, H * KD).astype(np.float32)
    Wv_f = Wv.reshape(D, H * KD).astype(np.float32)
    bq_f = bq.reshape(H * KD).astype(np.float32)
    bv_f = bv.reshape(H * KD).astype(np.float32)
    bv_eff = bv_ln.astype(np.float32) @ Wv_f + bv_f
    const_full = sum(
        bv_eff[h * KD:(h + 1) * KD] @ Wo[h].astype(np.float32) for h in range(H)
    ) + bo.astype(np.float32)

    def chunked8(w_eff):
        return np.ascontiguousarray(
            (16.0 * w_eff).reshape(2, 128, HPC * KD).transpose(1, 0, 2)
        ).astype(f8)

    in_maps = []
    for c in range(NCORES):
        b, hg = c // 2, c % 2
        hsl = slice(hg * HPC * KD, (hg + 1) * HPC * KD)
        wq8 = chunked8(gq[:, None] * Wq_f[:, hsl])
        wk8 = chunked8(gk[:, None] * Wk_f[:, hsl])
        wv8 = chunked8(gv[:, None] * Wv_f[:, hsl])
        bq_eff = bq_ln.astype(np.float32) @ Wq_f[:, hsl] + bq_f[hsl]
        bqt_np = np.ascontiguousarray(
            bq_eff.reshape(2 * HPC, 128).T.astype(np.float32))
        wo_np = np.ascontiguousarray(
            Wo[hg * HPC:(hg + 1) * HPC].astype(np.float32)
            .reshape(HPC, 2, 128, D).transpose(2, 1, 0, 3)).astype(bf)
        in_maps.append({
            "xq": np.ascontiguousarray(input_query[b], np.float32),
            "xk": np.ascontiguousarray(key[b], np.float32),
            "xv": np.ascontiguousarray(value[b], np.float32),
            "wq": wq8, "wk": wk8, "wv": wv8,
            "wo": wo_np, "bqt": bqt_np,
        })
    return in_maps, const_full


def kernel(_trace=False, **inputs):
    inputs = {k: np.asarray(v) for k, v in inputs.items()}
    in_maps, const_full = _host_prep(**inputs)
    nc = _get_program()
    res = run_bass_kernel_spmd(nc, in_maps, core_ids=list(range(NCORES)),
                               trace=_trace)
    x_q = inputs["input_query"].astype(np.float32)
    out = np.empty((B, S, D), np.float32)
    for b in range(B):
        # device out: [NQ, 128(p=do%128), 2(dc), 512(col)] ->
        #   out[qb*512+col, dc*128+p]
        a0 = res.results[2 * b]["out"].astype(np.float32)
        a1 = res.results[2 * b + 1]["out"].astype(np.float32)
        part = (a0 + a1).transpose(0, 3, 2, 1).reshape(S, D)
        out[b] = x_q[b] + const_full[None, :] + part
    if _trace:
        return out, res
    return out
